# revision 1
# baseline (speedup 1.0000x reference)
"""GCN regressor (3x GCNConv + BatchNorm + ReLU) on 8 Trainium2 NeuronCores.

Sharding (graph/data parallel, per the hint):
  - Nodes are split into 8 contiguous blocks of 6250; a core owns the dsts in
    its block and all edges pointing at them. Within a core, local dsts are
    sorted by in-degree (host) and laid out rank i -> (p=i%128, t=i//128) in a
    [128, 49*64] SBUF accumulator (padded to 6272 rows).
  - Per layer the dinv-prescaled feature table (all 50176 padded rows) is
    AllGathered into every core's DRAM (the "halo exchange"). The sparse
    aggregation sum_{s in N(d)} dinv[s]*h[s] runs as canonical indirect-DMA
    gather-accumulate instructions: 128 rows per instruction (offsets [128,1],
    dest [128,64]) with CCE add into SBUF; round r of slot t covers the r-th
    in-edge of that slot's 128 dsts; pad lanes target a guaranteed-zero table
    row. K=4 accumulator chains overlap DMA latency.
  - Dense math (dinv scaling, self loop, x@W, BN, ReLU) runs on PE/DVE/ACT;
    BN statistics are AllReduced (biased variance, as the reference).
  - b1/b2 are absorbed by BatchNorm (BN(z+b)=BN(z)) and unused.

kernel(**inputs) takes FULL inputs, returns the FULL [50000] output (f32).
"""

import sys

sys.path.insert(0, '/opt/trn_rl_repo')

import numpy as np

import concourse.bass as bass
import concourse.bacc as bacc
import concourse.tile as tile
import concourse.mybir as mybir
from concourse.masks import make_identity

F32 = mybir.dt.float32
AF = mybir.ActivationFunctionType

D = 64
NC = 8
EPS = 1e-5


class Cfg:
    def __init__(self, n_nodes, n_cores=NC, kchain=4):
        self.n = n_nodes
        self.nc = n_cores
        self.nloc = n_nodes // n_cores
        assert self.nloc * n_cores == n_nodes
        self.slots = self.nloc // 128 + 1          # ensures pad ranks exist
        self.npad = self.slots * 128
        assert self.nloc < self.npad
        self.ntab = self.npad * n_cores
        self.kchain = kchain


def host_prep(cfg, edge_index):
    """Host-side index work: degrees, per-core degree-sort, gather schedule."""
    n, nc_, nloc, npad, S = cfg.n, cfg.nc, cfg.nloc, cfg.npad, cfg.slots
    src = np.asarray(edge_index[0], dtype=np.int64)
    dst = np.asarray(edge_index[1], dtype=np.int64)
    deg = np.bincount(dst, minlength=n).astype(np.int64) + 1   # + self loop

    perms = []
    rank = np.zeros(n, dtype=np.int64)
    for c in range(nc_):
        nodes = np.arange(c * nloc, (c + 1) * nloc)
        p = nodes[np.argsort(-deg[nodes], kind="stable")]
        perms.append(p)
        rank[p] = np.arange(nloc)

    # table row of node m: core*npad + (rank%128)*S + rank//128
    g_of = (np.int64(npad) * (np.arange(n) // nloc)
            + (rank % 128) * S + rank // 128)
    zrow = 127 * S + (S - 1)          # pad rank npad-1 of core 0 (zero row)

    per_core = []
    max_rounds = np.zeros(S, dtype=np.int64)
    for c in range(nc_):
        m = (dst // nloc) == c
        s_c, r_c = src[m], rank[dst[m]]
        o = np.argsort(r_c, kind="stable")
        s_c, r_c = s_c[o], r_c[o]
        cnt = np.bincount(r_c, minlength=npad)
        starts = np.concatenate([[0], np.cumsum(cnt)])
        per_core.append((s_c, cnt, starts))
        for t in range(S):
            max_rounds[t] = max(max_rounds[t], cnt[t * 128:(t + 1) * 128].max())

    sched = np.concatenate(
        [np.full(int(max_rounds[t]), t, dtype=np.int64) for t in range(S)]
    ) if max_rounds.sum() else np.zeros(0, np.int64)
    ni = len(sched)

    offs = np.zeros((nc_, 128, max(ni, 1)), dtype=np.int32)
    rr = np.arange(128)
    for c in range(nc_):
        s_c, cnt, starts = per_core[c]
        col = np.zeros(S, dtype=np.int64)
        off = np.full((128, max(ni, 1)), zrow, dtype=np.int64)
        for j, t in enumerate(sched):
            r = col[t]
            col[t] += 1
            ranks = t * 128 + rr
            have = cnt[ranks] > r
            off[rr[have], j] = g_of[s_c[starts[ranks[have]] + r]]
        offs[c] = off.astype(np.int32)

    return deg, perms, zrow, sched, offs, ni


def build(cfg, ni, sched):
    nc = bacc.Bacc("TRN2", target_bir_lowering=False, debug=False,
                   enable_asserts=False, num_devices=cfg.nc,
                   num_swdge_queues=4)
    S = cfg.slots
    NPF = S * 64
    NT = cfg.ntab
    NP = cfg.npad

    xl = nc.dram_tensor("xl", [128, NPF], F32, kind="ExternalInput").ap()
    degt = nc.dram_tensor("degt", [128, S], F32, kind="ExternalInput").ap()
    off = nc.dram_tensor("off", [128, max(ni, 1)], mybir.dt.int32,
                         kind="ExternalInput").ap()
    w1 = nc.dram_tensor("w1", [D, D], F32, kind="ExternalInput").ap()
    w2 = nc.dram_tensor("w2", [D, D], F32, kind="ExternalInput").ap()
    w3 = nc.dram_tensor("w3", [D, 1], F32, kind="ExternalInput").ap()
    gb = nc.dram_tensor("gb", [4, D], F32, kind="ExternalInput").ap()
    b3 = nc.dram_tensor("b3", [1, 1], F32, kind="ExternalInput").ap()
    out = nc.dram_tensor("out", [1, NP], F32, kind="ExternalOutput").ap()

    inv_n = 1.0 / cfg.n

    with tile.TileContext(nc) as tc:
        with tc.tile_pool(name="const", bufs=1) as cpool, \
             tc.tile_pool(name="work", bufs=1) as wpool, \
             tc.tile_pool(name="ps", bufs=2, space="PSUM") as ppool, \
             tc.tile_pool(name="ps1", bufs=1, space="PSUM") as ppool1, \
             tc.tile_pool(name="dram", bufs=1, space="DRAM") as dpool:

            ident = cpool.tile([128, 128], F32)
            make_identity(nc, ident[:])
            w1t = cpool.tile([D, D], F32)
            w2t = cpool.tile([D, D], F32)
            w3t = cpool.tile([D, 1], F32)
            gbt = cpool.tile([4, D], F32)
            b3t = cpool.tile([1, 1], F32)
            epst = cpool.tile([D, 1], F32)
            nc.sync.dma_start(w1t[:], w1[:])
            nc.sync.dma_start(w2t[:], w2[:])
            nc.sync.dma_start(w3t[:], w3[:])
            nc.sync.dma_start(gbt[:], gb[:])
            nc.sync.dma_start(b3t[:], b3[:])
            nc.vector.memset(epst[:], EPS)
            offt = cpool.tile([128, max(ni, 1)], mybir.dt.int32)
            nc.sync.dma_start(offt[:], off[:])
            degs = cpool.tile([128, S], F32)
            nc.sync.dma_start(degs[:], degt[:])

            # gbT [64, 4] = (g1, bt1, g2, bt2) columns
            pgb = ppool1.tile([D, 4], F32, name="pgb")
            nc.tensor.transpose(pgb[:], gbt[:], ident[:4, :4])
            gbs = cpool.tile([D, 4], F32)
            nc.vector.tensor_copy(out=gbs[:], in_=pgb[:])

            dinv = cpool.tile([128, S], F32)
            nc.scalar.sqrt(dinv[:], degs[:])
            nc.vector.reciprocal(dinv[:], dinv[:])
            dinv_exp = cpool.tile([128, NPF], F32)
            for t in range(S):
                nc.vector.tensor_copy(
                    out=dinv_exp[:, t * 64:(t + 1) * 64],
                    in_=dinv[:, t:t + 1].to_broadcast([128, 64]))

            hloc = cpool.tile([128, NPF], F32)
            xin = wpool.tile([128, NPF], F32, name="xin")
            nc.sync.dma_start(xin[:], xl[:])
            nc.vector.tensor_mul(out=hloc[:], in0=xin[:], in1=dinv_exp[:])

            ag_in = dpool.tile([NP, D], F32, name="ag_in")
            tabs = [dpool.tile([NT, D], F32, name=f"tab{l}") for l in range(3)]
            ar_in = dpool.tile([D, 2], F32, name="ar_in")
            ar_out = [dpool.tile([D, 2], F32, name=f"ar_out{l}")
                      for l in range(2)]

            chains = [cpool.tile([128, NPF], F32, name=f"chain{k}")
                      for k in range(cfg.kchain)]
            yT = cpool.tile([D, NP], F32, name="yT")
            zT = cpool.tile([D, NP], F32, name="zT")
            st = cpool.tile([D, 2], F32, name="st")
            stg = cpool.tile([D, 2], F32, name="stg")
            scb = cpool.tile([D, 4], F32, name="scb")
            msq = cpool.tile([D, 1], F32, name="msq")
            rstd = cpool.tile([D, 1], F32, name="rstd")

            for layer in range(3):
                # publish local slice; AllGather the layer's table
                # (HWDGE, not gpsimd: keeps the Pool engine free for SWDGE
                # descriptor generation, which is the kernel's bottleneck)
                nc.sync.dma_start(
                    ag_in.rearrange("(p t) f -> p (t f)", t=S), hloc[:])
                if cfg.nc > 1:
                    nc.gpsimd.collective_compute(
                        "AllGather", mybir.AluOpType.bypass,
                        replica_groups=[list(range(cfg.nc))],
                        ins=[ag_in.opt()], outs=[tabs[layer].opt()],
                    )
                else:  # single-core timing/sim stub
                    nc.sync.dma_start(tabs[layer][0:NP, :], ag_in[:])
                tab = tabs[layer]

                # Rounds 0..kchain-1 of each slot land on the kchain distinct
                # chains (j cycles chains, a slot's rounds are consecutive j),
                # so when every slot has >= kchain rounds the first touch per
                # (chain, slot) can be a bypass (overwrite) gather: no memsets,
                # no chain-head completion waits. Pad lanes read the zero row,
                # which zeroes them exactly as memset did.
                first_touch_init = bool(np.min(
                    np.bincount(sched, minlength=S)) >= cfg.kchain) if ni else False
                if not first_touch_init:
                    for k in range(cfg.kchain):
                        nc.vector.memset(chains[k][:], 0.0)
                # chain k rides SWDGE queue k: same-chain FIFO order is kept
                # within one queue, and on HW the 4 queue contexts can
                # generate descriptors concurrently on the Q7 cluster.
                qnames = ("qPoolDynamic", "qPoolDynamic1",
                          "qPoolDynamic2", "qPoolDynamic3")
                rounds_seen = [0] * S
                for j in range(ni):
                    t = int(sched[j])
                    r = rounds_seen[t]
                    rounds_seen[t] += 1
                    op = (mybir.AluOpType.bypass
                          if first_touch_init and r < cfg.kchain
                          else mybir.AluOpType.add)
                    bi = nc.gpsimd.indirect_dma_start(
                        out=chains[j % cfg.kchain][:, t * 64:(t + 1) * 64],
                        out_offset=None,
                        in_=tab[:],
                        in_offset=bass.IndirectOffsetOnAxis(
                            ap=offt[:, j:j + 1], axis=0),
                        compute_op=op,
                    )
                    bi.ins.queue = qnames[j % cfg.kchain]
                acc = chains[0]
                nc.vector.tensor_add(out=acc[:], in0=acc[:], in1=chains[1][:])
                nc.vector.tensor_add(out=chains[2][:], in0=chains[2][:],
                                     in1=chains[3][:])
                nc.vector.tensor_add(out=acc[:], in0=acc[:], in1=chains[2][:])

                # y = dinv * (acc + hloc)   [node-major]
                nc.vector.tensor_add(out=acc[:], in0=acc[:], in1=hloc[:])
                nc.vector.tensor_mul(out=acc[:], in0=acc[:], in1=dinv_exp[:])

                # transpose to f-major yT [64, NP]
                for t in range(S):
                    pt = ppool.tile([D, 128], F32, tag="tp", name="pt")
                    nc.tensor.transpose(pt[:], acc[:, t * 64:(t + 1) * 64],
                                        ident[:])
                    nc.vector.tensor_copy(out=yT[:, t * 128:(t + 1) * 128],
                                          in_=pt[:])

                if layer < 2:
                    w_cur = (w1t, w2t)[layer]
                    for k in range((NP + 511) // 512):
                        c0, c1 = k * 512, min(NP, k * 512 + 512)
                        pz = ppool.tile([D, 512], F32, tag="pz", name="pz")
                        nc.tensor.matmul(pz[:, :c1 - c0], w_cur[:],
                                         yT[:, c0:c1], start=True, stop=True)
                        nc.vector.tensor_copy(out=zT[:, c0:c1],
                                              in_=pz[:, :c1 - c0])
                    # local BN sums: sum(z) and sum(z^2)  (pads are zero)
                    nc.scalar.activation(yT[:], zT[:], AF.Copy,
                                         accum_out=st[:, 0:1])
                    nc.scalar.activation(yT[:], zT[:], AF.Square,
                                         accum_out=st[:, 1:2])
                    nc.sync.dma_start(ar_in[:], st[:])
                    if cfg.nc > 1:
                        nc.gpsimd.collective_compute(
                            "AllReduce", mybir.AluOpType.add,
                            replica_groups=[list(range(cfg.nc))],
                            ins=[ar_in.opt()], outs=[ar_out[layer].opt()],
                        )
                    else:
                        nc.sync.dma_start(ar_out[layer][:], ar_in[:])
                    nc.sync.dma_start(stg[:], ar_out[layer][:])
                    nc.scalar.mul(scb[:, 0:1], stg[:, 0:1], inv_n)
                    nc.scalar.mul(scb[:, 1:2], stg[:, 1:2], inv_n)
                    nc.vector.tensor_mul(out=msq[:], in0=scb[:, 0:1],
                                         in1=scb[:, 0:1])
                    nc.vector.tensor_sub(out=scb[:, 1:2], in0=scb[:, 1:2],
                                         in1=msq[:])
                    # rstd = 1/sqrt(var+eps)
                    nc.scalar.activation(rstd[:], scb[:, 1:2], AF.Sqrt,
                                         bias=epst[:, 0:1])
                    nc.vector.reciprocal(rstd[:], rstd[:])
                    nc.vector.tensor_mul(out=scb[:, 2:3],
                                         in0=gbs[:, 2 * layer:2 * layer + 1],
                                         in1=rstd[:])
                    nc.vector.tensor_mul(out=msq[:], in0=scb[:, 0:1],
                                         in1=scb[:, 2:3])
                    nc.vector.tensor_sub(out=scb[:, 3:4],
                                         in0=gbs[:, 2 * layer + 1:2 * layer + 2],
                                         in1=msq[:])
                    # h.T = Relu(scale*z + bias); hloc = dinv * h (node-major)
                    nc.scalar.activation(yT[:], zT[:], AF.Relu,
                                         bias=scb[:, 3:4], scale=scb[:, 2:3])
                    for t in range(S):
                        ph = ppool.tile([128, D], F32, tag="tp", name="ph")
                        nc.tensor.transpose(ph[:], yT[:, t * 128:(t + 1) * 128],
                                            ident[:64, :64])
                        nc.vector.tensor_mul(
                            out=hloc[:, t * 64:(t + 1) * 64], in0=ph[:],
                            in1=dinv_exp[:, t * 64:(t + 1) * 64])
                    # pad rows self-zero: their dinv is ~1e-19 (deg=1e38)
                else:
                    o_s = wpool.tile([1, NP], F32, name="o_s")
                    for k in range((NP + 511) // 512):
                        c0, c1 = k * 512, min(NP, k * 512 + 512)
                        po = ppool.tile([1, 512], F32, tag="pz", name="po")
                        nc.tensor.matmul(po[:, :c1 - c0], w3t[:],
                                         yT[:, c0:c1], start=True, stop=True)
                        nc.scalar.add(o_s[:, c0:c1], po[:, :c1 - c0],
                                      add=b3t[:])
                    nc.sync.dma_start(out[:], o_s[:])

    nc.compile()
    return nc


def make_in_maps(cfg, inputs, deg, perms, offs):
    x = np.asarray(inputs["x"], dtype=np.float32)
    in_maps = []
    for c in range(cfg.nc):
        xp = np.zeros((cfg.npad, D), np.float32)
        xp[:cfg.nloc] = x[perms[c]]
        xlc = xp.reshape(cfg.slots, 128, D).transpose(1, 0, 2).reshape(128, -1)
        dg = np.full((cfg.npad,), 1e30, np.float32)
        dg[:cfg.nloc] = deg[perms[c]].astype(np.float32)
        dgt = dg.reshape(cfg.slots, 128).T.copy()
        in_maps.append({
            "xl": np.ascontiguousarray(xlc),
            "degt": np.ascontiguousarray(dgt),
            "off": np.ascontiguousarray(offs[c]),
            "w1": np.asarray(inputs["W1"], np.float32),
            "w2": np.asarray(inputs["W2"], np.float32),
            "w3": np.asarray(inputs["W3"], np.float32).reshape(D, 1),
            "gb": np.stack([
                np.asarray(inputs["g1"], np.float32),
                np.asarray(inputs["bt1"], np.float32),
                np.asarray(inputs["g2"], np.float32),
                np.asarray(inputs["bt2"], np.float32)]),
            "b3": np.asarray(inputs["b3"], np.float32).reshape(1, 1),
        })
    return in_maps


_CACHE = {}


def kernel(**inputs):
    cfg = Cfg(n_nodes=int(np.asarray(inputs["x"]).shape[0]), n_cores=NC)
    deg, perms, zrow, sched, offs, ni = host_prep(
        cfg, np.asarray(inputs["edge_index"]))

    key = (cfg.n, ni, sched.tobytes())
    if key not in _CACHE:
        _CACHE[key] = build(cfg, ni, sched)
    nc = _CACHE[key]
    in_maps = make_in_maps(cfg, inputs, deg, perms, offs)

    import concourse.bass_utils as bass_utils
    res = None
    for attempt in range(3):
        try:
            res = bass_utils.run_bass_kernel_spmd(
                nc, in_maps, core_ids=list(range(cfg.nc)))
            break
        except Exception:
            if attempt == 2:
                raise
    out = np.zeros((cfg.n,), np.float32)
    for c in range(cfg.nc):
        oc = np.asarray(res.results[c]["out"]).reshape(cfg.npad)
        out[perms[c]] = oc[:cfg.nloc]
    return out

